# revision 34
# baseline (speedup 1.0000x reference)
"""Bass/Trainium2 kernel for BNBLinear4bit (NF4 dequant + matmul + bias).

Strategy (8 NeuronCores, tensor-parallel on out_features):
  - out_features sharded 8 ways (512 rows of codes/absmax/bias per core);
    x replicated: each core streams all 32 bs-tiles of x, casting
    f32->fp16 in-DMA and xbar-transposing each [128, 4096] tile straight
    into SBUF for the PE (no DRAM bounce, no collectives - the 8 "cores"
    are 4 devices x 2 and cross-device collectives cost ~60us fixed).
  - NF4 dequant via a degree-7 minimax polynomial in u=(c-7.5)/7.5
    (max residual 0.0098, inside the 2e-2 gate): ACT computes u, u^2,
    u^4 (in-DMA int32->int8 codes); DVE evaluates Estrin (4
    tensor_scalar at 4x rate + 6 tensor_tensor at 2x); Pool applies the
    per-64-block absmax; ACT xbar-transposes w into [i, o] fp16.
  - matmul: stationary x^T tile [128i,128bs], moving w^T [128i,512o]
    full width, fp16 at full PE rate (LDWEIGHTS overlaps MATMUL), fp32
    PSUM accumulated over all 32 k-tiles; i-half dequant order lets the
    first 16 k matmuls start while the second half still dequantizes.
  - DVE evacuates PSUM fused with the bias add; host-side probe check
    catches the (rare) flaky-core run and retries.
"""
import sys

sys.path.insert(0, "/opt/trn_rl_repo")

import numpy as np

import concourse.bass as bass
import concourse.mybir as mybir
from concourse import bacc
from concourse.bass_utils import run_bass_kernel_spmd
from concourse.tile import TileContext

F16 = mybir.dt.float16
F32 = mybir.dt.float32
I8 = mybir.dt.int8
ALU = mybir.AluOpType
ACTF = mybir.ActivationFunctionType

NF4 = np.array([
    -1.0, -0.6961928009986877, -0.5250730514526367, -0.39491748809814453,
    -0.28444138169288635, -0.18477343022823334, -0.09105003625154495, 0.0,
    0.07958029955625534, 0.16093020141124725, 0.24611230194568634,
    0.33791524171829224, 0.44070982933044434, 0.5626170039176941,
    0.6797559261322021, 1.0], dtype=np.float64)

BLOCKSIZE = 64
N_CORES = 8
P = 128


def _fit_poly(deg=7):
    """Minimax-ish poly fit of NF4[c] in u=(c-7.5)/7.5 on the 16 codes."""
    c = np.arange(16.0)
    u = (c - 7.5) / 7.5
    A = np.stack([u ** j for j in range(deg + 1)], axis=1)
    w = np.ones(16)
    coef = None
    for _ in range(300):
        W = np.sqrt(w)[:, None]
        coef, *_ = np.linalg.lstsq(A * W, NF4 * np.sqrt(w), rcond=None)
        r = np.abs(A @ coef - NF4)
        w *= (1e-12 + r)
        w /= w.sum()
    return [float(v) for v in coef]


def build_bass(BS, IN, OSH, n_cores=N_CORES):
    """Per-core Bass program, run SPMD on all cores."""
    KT = IN // P              # contraction k-tiles
    OPT = OSH // P            # o partition-tiles of the codes slice
    IH = IN // 2              # dequant chunk width
    KH = KT // 2              # k-tiles per dequant i-half
    NBH = IH // BLOCKSIZE     # absmax blocks per i-half
    SL = BS // n_cores        # bs rows per core slice
    AT = SL // (2 * P)        # A-half tiles staged per core for the AG (2)
    NBT = BS // (2 * P)       # B-half tiles self-staged by every core (16)

    a = _fit_poly(7)

    nc = bacc.Bacc(trn_type="TRN2", num_devices=n_cores)
    # xa: this core's A-half rows (first 256 of its 512-row slice)
    xa_d = nc.dram_tensor("xa", [AT * P, IN], F32, kind="ExternalInput")
    # xb: B-half rows of every slice (same array on every core)
    xb_d = nc.dram_tensor("xb", [NBT * P, IN], F32, kind="ExternalInput")
    codes_d = nc.dram_tensor("codes", [OSH, IN], mybir.dt.int32,
                             kind="ExternalInput")
    absmax_d = nc.dram_tensor("absmax", [OSH, IN // BLOCKSIZE], F32,
                              kind="ExternalInput")
    bias_d = nc.dram_tensor("bias", [OSH], F32, kind="ExternalInput")
    out_d = nc.dram_tensor("out", [BS, OSH], F32, kind="ExternalOutput")

    with TileContext(nc) as tc:
        with (
            tc.tile_pool(name="const", bufs=1) as const_pool,
            tc.tile_pool(name="xn", bufs=2) as xn_pool,
            tc.tile_pool(name="xa", bufs=1) as xa_pool,
            tc.tile_pool(name="wt", bufs=1) as wt_pool,
            tc.tile_pool(name="c8", bufs=1) as c8_pool,
            tc.tile_pool(name="u", bufs=2) as u_pool,
            tc.tile_pool(name="u2", bufs=2) as u2_pool,
            tc.tile_pool(name="u4", bufs=2) as u4_pool,
            tc.tile_pool(name="L", bufs=1) as L_pool,
            tc.tile_pool(name="M", bufs=1) as M_pool,
            tc.tile_pool(name="wn", bufs=2) as wn_pool,
            tc.tile_pool(name="xts", bufs=4) as xts_pool,
            tc.tile_pool(name="xb", bufs=2) as xb_pool,
            tc.tile_pool(name="osb", bufs=2) as osb_pool,
            tc.tile_pool(name="dram", bufs=1, space="DRAM") as dram,
            tc.tile_pool(name="psum", bufs=4, space="PSUM") as psum_pool,
        ):
            # ---- A-half staging for the AllGather: cast + xbar own rows
            xt_a = xa_pool.tile([P, AT, KT, P], F16)
            for t in range(AT):
                xn = xn_pool.tile([P, IN], F16, name="xn")
                nc.gpsimd.dma_start(xn[:], xa_d[t * P:(t + 1) * P, :])
                nc.sync.dma_start_transpose(xt_a[:, t, :, :], xn[:])

            # ---- codes + consts loads ----
            c8s = []
            xbns = [None] * (NBT // 2)

            def load_xb(g):
                if g >= NBT // 2:
                    return
                xn2 = xb_pool.tile([P, 2, IN], F16, name="xb")
                nc.gpsimd.dma_start(
                    xn2[:],
                    xb_d[g * 2 * P:(g + 1) * 2 * P, :]
                    .rearrange("(t p) i -> p t i", p=P))
                xbns[g] = xn2

            for ih in range(2):
                for op in range(OPT):
                    c8 = c8_pool.tile([P, IH], I8, tag=f"c8_{ih}_{op}",
                                      name="c8")
                    nc.gpsimd.dma_start(
                        c8[:], codes_d[op * P:(op + 1) * P,
                                       ih * IH:(ih + 1) * IH])
                    c8s.append(c8)
            amax = []
            for op in range(OPT):
                t = const_pool.tile([P, IN // BLOCKSIZE], F32,
                                    tag=f"amax{op}", name="am")
                nc.sync.dma_start(t[:], absmax_d[op * P:(op + 1) * P, :])
                amax.append(t)
            brep = const_pool.tile([P, OSH], F32)
            nc.gpsimd.dma_start(brep[:], bias_d[None, :].broadcast_to([P, OSH]))

            # ---- single AllGather of the A-halves (one per core) ----
            xtb = dram.tile([P, AT, KT, P], F16)
            nc.sync.dma_start(xtb[:], xt_a[:])
            xg = dram.tile([n_cores, P, AT, KT, P], F16)

            def emit_ag():
                nc.gpsimd.collective_compute(
                    "AllGather",
                    ALU.bypass,
                    replica_groups=[list(range(n_cores))],
                    ins=[xtb.opt()],
                    outs=[xg.opt()],
                )

            # ---- dequant: poly in u = (c-7.5)/7.5, Estrin on DVE ----
            # w^T fp16 [P, KT*OSH]; element (p, k*OSH + o) = w[o, k*P + p]
            wT = wt_pool.tile([P, KT * OSH], F16)
            wT3 = wT[:].rearrange("p (k o) -> p k o", k=KT)

            pend_xbar = []  # (wn, ih, op): emitted one chunk late on scalar

            def emit_xbar():
                wn, xih, xop = pend_xbar.pop(0)
                nc.scalar.dma_start_transpose(
                    wT3[:, xih * KH:(xih + 1) * KH, xop * P:(xop + 1) * P],
                    wn[:])

            for ih in range(2):
                for op in range(OPT):
                    ci = ih * OPT + op
                    c8 = c8s[ci]
                    u = u_pool.tile([P, IH], F16, name="u")
                    nc.scalar.activation(u[:], c8[:], ACTF.Copy,
                                         bias=-1.0, scale=1.0 / 7.5)
                    u2 = u2_pool.tile([P, IH], F16, name="u2")
                    nc.scalar.activation(u2[:], u[:], ACTF.Square)
                    u4 = u4_pool.tile([P, IH], F16, name="u4")
                    nc.scalar.activation(u4[:], u2[:], ACTF.Square)
                    if pend_xbar:
                        emit_xbar()
                    # DVE: L_j = a[2j+1]*u + a[2j]
                    L = [L_pool.tile([P, IH], F16, name=f"L{j}")
                         for j in range(4)]
                    for j in range(4):
                        nc.vector.tensor_scalar(
                            L[j][:], u[:], a[2 * j + 1], a[2 * j],
                            ALU.mult, ALU.add)
                    M0 = M_pool.tile([P, IH], F16, name="M0")
                    nc.vector.tensor_mul(M0[:], L[1][:], u2[:])
                    nc.vector.tensor_add(M0[:], M0[:], L[0][:])
                    M1 = M_pool.tile([P, IH], F16, name="M1")
                    nc.vector.tensor_mul(M1[:], L[3][:], u2[:])
                    nc.vector.tensor_add(M1[:], M1[:], L[2][:])
                    nc.vector.tensor_mul(M1[:], M1[:], u4[:])
                    nc.vector.tensor_add(M1[:], M1[:], M0[:])
                    # scale by absmax (per 64-block) on Pool -> wn
                    wn = wn_pool.tile([P, IH], F16, name="wn")
                    nc.gpsimd.tensor_mul(
                        wn[:].rearrange("p (b r) -> p b r", b=NBH),
                        M1[:].rearrange("p (b r) -> p b r", b=NBH),
                        amax[op][:, ih * NBH:(ih + 1) * NBH][:, :, None]
                        .broadcast_to([P, NBH, BLOCKSIZE]))
                    pend_xbar.append((wn, ih, op))
                    # interleave B-half x loads and the AG trigger on the
                    # gpsimd queue behind this chunk's Pool multiply
                    load_xb(ci)
                    if ci == 1:
                        emit_ag()
            while pend_xbar:
                emit_xbar()

            # ---- matmul helpers ----
            def mm_tile(xts_ap, rows):
                ps = psum_pool.tile([P, OSH], F32, name="ps")
                for k in range(KT):
                    nc.tensor.matmul(
                        ps[:],
                        xts_ap[:, k, :],
                        wT3[:, k, :],
                        start=(k == 0), stop=(k == KT - 1))
                osb = osb_pool.tile([P, OSH], F32, name="osb")
                nc.vector.tensor_add(osb[:], ps[:], brep[:])
                nc.scalar.dma_start(out_d[rows:rows + P, :], osb[:])

            def mm_b(g, t):
                xts = xts_pool.tile([P, KT, P], F16, name="xts")
                nc.sync.dma_start_transpose(
                    xts[:], xbns[g][:, t, :])
                c, tt = (2 * g + t) // 2, (2 * g + t) % 2
                mm_tile(xts[:], c * SL + (SL // 2) + tt * P)

            # B-tiles from the first 4 groups while the AG is in flight
            for g in range(4):
                for t in range(2):
                    mm_b(g, t)
            # gathered A-tiles (PE-paced reads, 2.9us each)
            for c in range(n_cores):
                for t in range(AT):
                    xts = xts_pool.tile([P, KT, P], F16, name="xts")
                    nc.sync.dma_start(xts[:], xg[c, :, t, :, :])
                    mm_tile(xts[:], c * SL + t * P)
            # remaining B-tiles
            for g in range(4, NBT // 2):
                for t in range(2):
                    mm_b(g, t)

    nc.compile()
    nc.finalize()
    return nc


_CACHE = {}
TRACE = False
LAST_EXEC_NS = None
LAST_RES = None


def _get_nc():
    if "nc" not in _CACHE:
        _CACHE["nc"] = build_bass(4096, 4096, 512)
    return _CACHE["nc"]


def _probe_check(out, xf, codes, absmax, bias, rng):
    """Cheap host check: one random bs row per core shard vs exact math."""
    BS, IN = xf.shape
    OSH = out.shape[1] // N_CORES
    scale = np.repeat(absmax.astype(np.float64), BLOCKSIZE, axis=1)
    for c in range(N_CORES):
        r = int(rng.integers(0, BS))
        osl = slice(c * OSH, (c + 1) * OSH)
        w = NF4[codes[osl]] * scale[osl]          # [OSH, IN] f64
        exp = w @ xf[r].astype(np.float64) + bias[osl]
        err = np.abs(out[r, osl] - exp).max()
        if not (err < 5.0):  # catches NaN too
            return False, c, err
    return True, -1, 0.0


def kernel(x, codes, absmax, bias):
    x = np.ascontiguousarray(np.asarray(x, dtype=np.float32))
    codes = np.ascontiguousarray(np.asarray(codes, dtype=np.int32))
    absmax = np.ascontiguousarray(np.asarray(absmax, dtype=np.float32))
    bias = np.ascontiguousarray(np.asarray(bias, dtype=np.float32))

    B, S, IN = x.shape
    OUT = codes.shape[0]
    BS = B * S
    OSH = OUT // N_CORES
    xf = np.ascontiguousarray(x.reshape(BS, IN))

    nc = _get_nc()
    SL = BS // N_CORES
    HA = SL // 2
    # xb: the second half of every core slice, shared by all cores
    xb = np.ascontiguousarray(
        xf.reshape(N_CORES, SL, IN)[:, HA:, :].reshape(BS // 2, IN))
    in_maps = []
    for c in range(N_CORES):
        osl = slice(c * OSH, (c + 1) * OSH)
        in_maps.append({
            "xa": np.ascontiguousarray(xf[c * SL:c * SL + HA]),
            "xb": xb,
            "codes": np.ascontiguousarray(codes[osl]),
            "absmax": np.ascontiguousarray(absmax[osl]),
            "bias": np.ascontiguousarray(bias[osl]),
        })
    global LAST_EXEC_NS, LAST_RES
    rng = np.random.default_rng(0)
    out = None
    for attempt in range(3):
        res = run_bass_kernel_spmd(nc, in_maps, core_ids=list(range(N_CORES)),
                                   trace=TRACE)
        LAST_EXEC_NS = res.exec_time_ns
        LAST_RES = res
        out = np.concatenate([res.results[c]["out"] for c in range(N_CORES)],
                             axis=1)
        ok, badcore, err = _probe_check(out, xf, codes, absmax, bias, rng)
        if ok:
            break
        print(f"kernel: probe check failed (core {badcore}, err {err:.1f}); "
              f"retrying ({attempt + 1}/3)", file=sys.stderr)
    return np.ascontiguousarray(out.reshape(B, S, OUT).astype(np.float32))


# revision 36
# speedup vs baseline: 1.0029x; 1.0029x over previous
"""Bass/Trainium2 kernel for BNBLinear4bit (NF4 dequant + matmul + bias).

Strategy (8 NeuronCores, tensor-parallel on out_features):
  - out_features sharded 8 ways (512 rows of codes/absmax/bias per core);
    x replicated: each core streams all 32 bs-tiles of x, casting
    f32->fp16 in-DMA and xbar-transposing each [128, 4096] tile straight
    into SBUF for the PE (no DRAM bounce, no collectives - the 8 "cores"
    are 4 devices x 2 and cross-device collectives cost ~60us fixed).
  - NF4 dequant via a degree-7 minimax polynomial in u=(c-7.5)/7.5
    (max residual 0.0098, inside the 2e-2 gate): ACT computes u, u^2,
    u^4 (in-DMA int32->int8 codes); DVE evaluates Estrin (4
    tensor_scalar at 4x rate + 6 tensor_tensor at 2x); Pool applies the
    per-64-block absmax; ACT xbar-transposes w into [i, o] fp16.
  - matmul: stationary x^T tile [128i,128bs], moving w^T [128i,512o]
    full width, fp16 at full PE rate (LDWEIGHTS overlaps MATMUL), fp32
    PSUM accumulated over all 32 k-tiles; i-half dequant order lets the
    first 16 k matmuls start while the second half still dequantizes.
  - DVE evacuates PSUM fused with the bias add; host-side probe check
    catches the (rare) flaky-core run and retries.
"""
import sys

sys.path.insert(0, "/opt/trn_rl_repo")

import numpy as np

import concourse.bass as bass
import concourse.mybir as mybir
from concourse import bacc
from concourse.bass_utils import run_bass_kernel_spmd
from concourse.tile import TileContext

F16 = mybir.dt.float16
F32 = mybir.dt.float32
I8 = mybir.dt.int8
ALU = mybir.AluOpType
ACTF = mybir.ActivationFunctionType

NF4 = np.array([
    -1.0, -0.6961928009986877, -0.5250730514526367, -0.39491748809814453,
    -0.28444138169288635, -0.18477343022823334, -0.09105003625154495, 0.0,
    0.07958029955625534, 0.16093020141124725, 0.24611230194568634,
    0.33791524171829224, 0.44070982933044434, 0.5626170039176941,
    0.6797559261322021, 1.0], dtype=np.float64)

BLOCKSIZE = 64
N_CORES = 8
P = 128


def _fit_poly(deg=7):
    """Minimax-ish poly fit of NF4[c] in u=(c-7.5)/7.5 on the 16 codes."""
    c = np.arange(16.0)
    u = (c - 7.5) / 7.5
    A = np.stack([u ** j for j in range(deg + 1)], axis=1)
    w = np.ones(16)
    coef = None
    for _ in range(300):
        W = np.sqrt(w)[:, None]
        coef, *_ = np.linalg.lstsq(A * W, NF4 * np.sqrt(w), rcond=None)
        r = np.abs(A @ coef - NF4)
        w *= (1e-12 + r)
        w /= w.sum()
    return [float(v) for v in coef]


def build_bass(BS, IN, OSH, n_cores=N_CORES):
    """Per-core Bass program, run SPMD on all cores."""
    KT = IN // P              # contraction k-tiles
    OPT = OSH // P            # o partition-tiles of the codes slice
    IH = IN // 2              # dequant chunk width
    KH = KT // 2              # k-tiles per dequant i-half
    NBH = IH // BLOCKSIZE     # absmax blocks per i-half
    SL = BS // n_cores        # bs rows per core slice
    AT = SL // (2 * P)        # A-half tiles staged per core for the AG (2)
    NBT = BS // (2 * P)       # B-half tiles self-staged by every core (16)

    a = _fit_poly(7)

    nc = bacc.Bacc(trn_type="TRN2", num_devices=n_cores)
    # xa: this core's A-half rows (first 256 of its 512-row slice)
    xa_d = nc.dram_tensor("xa", [AT * P, IN], F32, kind="ExternalInput")
    # xb: B-half rows of every slice (same array on every core)
    xb_d = nc.dram_tensor("xb", [NBT * P, IN], F32, kind="ExternalInput")
    codes_d = nc.dram_tensor("codes", [OSH, IN], mybir.dt.int32,
                             kind="ExternalInput")
    absmax_d = nc.dram_tensor("absmax", [OSH, IN // BLOCKSIZE], F32,
                              kind="ExternalInput")
    bias_d = nc.dram_tensor("bias", [OSH], F32, kind="ExternalInput")
    out_d = nc.dram_tensor("out", [BS, OSH], F32, kind="ExternalOutput")

    with TileContext(nc) as tc:
        with (
            tc.tile_pool(name="const", bufs=1) as const_pool,
            tc.tile_pool(name="xn", bufs=2) as xn_pool,
            tc.tile_pool(name="xa", bufs=1) as xa_pool,
            tc.tile_pool(name="wt", bufs=1) as wt_pool,
            tc.tile_pool(name="c8", bufs=1) as c8_pool,
            tc.tile_pool(name="u", bufs=2) as u_pool,
            tc.tile_pool(name="u2", bufs=2) as u2_pool,
            tc.tile_pool(name="u4", bufs=2) as u4_pool,
            tc.tile_pool(name="L", bufs=1) as L_pool,
            tc.tile_pool(name="M", bufs=1) as M_pool,
            tc.tile_pool(name="wn", bufs=2) as wn_pool,
            tc.tile_pool(name="xts", bufs=4) as xts_pool,
            tc.tile_pool(name="xb", bufs=2) as xb_pool,
            tc.tile_pool(name="osb", bufs=2) as osb_pool,
            tc.tile_pool(name="dram", bufs=1, space="DRAM") as dram,
            tc.tile_pool(name="psum", bufs=4, space="PSUM") as psum_pool,
        ):
            # ---- A-half staging for the AllGather: cast + xbar own rows
            xt_a = xa_pool.tile([P, AT, KT, P], F16)
            for t in range(AT):
                xn = xn_pool.tile([P, IN], F16, name="xn")
                nc.gpsimd.dma_start(xn[:], xa_d[t * P:(t + 1) * P, :])
                nc.sync.dma_start_transpose(xt_a[:, t, :, :], xn[:])

            # ---- codes + consts loads ----
            c8s = []
            xbns = [None] * (NBT // 2)

            def load_xb(g):
                if g >= NBT // 2:
                    return
                xn2 = xb_pool.tile([P, 2, IN], F16, name="xb")
                nc.gpsimd.dma_start(
                    xn2[:],
                    xb_d[g * 2 * P:(g + 1) * 2 * P, :]
                    .rearrange("(t p) i -> p t i", p=P))
                xbns[g] = xn2

            for ih in range(2):
                for op in range(OPT):
                    c8 = c8_pool.tile([P, IH], I8, tag=f"c8_{ih}_{op}",
                                      name="c8")
                    nc.gpsimd.dma_start(
                        c8[:], codes_d[op * P:(op + 1) * P,
                                       ih * IH:(ih + 1) * IH])
                    c8s.append(c8)
            amax = []
            for op in range(OPT):
                t = const_pool.tile([P, IN // BLOCKSIZE], F32,
                                    tag=f"amax{op}", name="am")
                nc.sync.dma_start(t[:], absmax_d[op * P:(op + 1) * P, :])
                amax.append(t)
            brep = const_pool.tile([P, OSH], F32)
            nc.gpsimd.dma_start(brep[:], bias_d[None, :].broadcast_to([P, OSH]))

            # ---- single AllGather of the A-halves (one per core) ----
            xtb = dram.tile([P, AT, KT, P], F16)
            nc.sync.dma_start(xtb[:], xt_a[:])
            xg = dram.tile([n_cores, P, AT, KT, P], F16)

            def emit_ag():
                nc.gpsimd.collective_compute(
                    "AllGather",
                    ALU.bypass,
                    replica_groups=[list(range(n_cores))],
                    ins=[xtb.opt()],
                    outs=[xg.opt()],
                )

            # ---- dequant: poly in u = (c-7.5)/7.5, Estrin on DVE ----
            # w^T fp16 [P, KT*OSH]; element (p, k*OSH + o) = w[o, k*P + p]
            wT = wt_pool.tile([P, KT * OSH], F16)
            wT3 = wT[:].rearrange("p (k o) -> p k o", k=KT)

            pend_xbar = []  # (wn, ih, op): emitted one chunk late on scalar

            def emit_xbar():
                wn, xih, xop = pend_xbar.pop(0)
                nc.scalar.dma_start_transpose(
                    wT3[:, xih * KH:(xih + 1) * KH, xop * P:(xop + 1) * P],
                    wn[:])

            for ih in range(2):
                for op in range(OPT):
                    ci = ih * OPT + op
                    c8 = c8s[ci]
                    u = u_pool.tile([P, IH], F16, name="u")
                    nc.scalar.activation(u[:], c8[:], ACTF.Copy,
                                         bias=-1.0, scale=1.0 / 7.5)
                    u2 = u2_pool.tile([P, IH], F16, name="u2")
                    nc.scalar.activation(u2[:], u[:], ACTF.Square)
                    u4 = u4_pool.tile([P, IH], F16, name="u4")
                    nc.scalar.activation(u4[:], u2[:], ACTF.Square)
                    if pend_xbar:
                        emit_xbar()
                    # DVE: L_j = a[2j+1]*u + a[2j]
                    L = [L_pool.tile([P, IH], F16, name=f"L{j}")
                         for j in range(4)]
                    for j in range(4):
                        nc.vector.tensor_scalar(
                            L[j][:], u[:], a[2 * j + 1], a[2 * j],
                            ALU.mult, ALU.add)
                    M0 = M_pool.tile([P, IH], F16, name="M0")
                    nc.vector.tensor_mul(M0[:], L[1][:], u2[:])
                    nc.vector.tensor_add(M0[:], M0[:], L[0][:])
                    M1 = M_pool.tile([P, IH], F16, name="M1")
                    nc.vector.tensor_mul(M1[:], L[3][:], u2[:])
                    nc.vector.tensor_add(M1[:], M1[:], L[2][:])
                    nc.vector.tensor_mul(M1[:], M1[:], u4[:])
                    nc.vector.tensor_add(M1[:], M1[:], M0[:])
                    # scale by absmax (per 64-block) on Pool -> wn
                    wn = wn_pool.tile([P, IH], F16, name="wn")
                    nc.gpsimd.tensor_mul(
                        wn[:].rearrange("p (b r) -> p b r", b=NBH),
                        M1[:].rearrange("p (b r) -> p b r", b=NBH),
                        amax[op][:, ih * NBH:(ih + 1) * NBH][:, :, None]
                        .broadcast_to([P, NBH, BLOCKSIZE]))
                    pend_xbar.append((wn, ih, op))
                    if ci == 1:
                        emit_ag()
            while pend_xbar:
                emit_xbar()

            # ---- matmul helpers ----
            def mm_tile(xts_ap, rows):
                ps = psum_pool.tile([P, OSH], F32, name="ps")
                for k in range(KT):
                    nc.tensor.matmul(
                        ps[:],
                        xts_ap[:, k, :],
                        wT3[:, k, :],
                        start=(k == 0), stop=(k == KT - 1))
                osb = osb_pool.tile([P, OSH], F32, name="osb")
                nc.vector.tensor_add(osb[:], ps[:], brep[:])
                nc.scalar.dma_start(out_d[rows:rows + P, :], osb[:])

            def mm_b(g, t):
                if t == 0:
                    load_xb(g)
                xts = xts_pool.tile([P, KT, P], F16, name="xts")
                nc.sync.dma_start_transpose(
                    xts[:], xbns[g][:, t, :])
                mm_tile(xts[:], g * SL + (SL // 2) + t * P)

            # B-tiles from the first 4 groups while the AG is in flight
            for g in range(4):
                for t in range(2):
                    mm_b(g, t)
            # gathered A-tiles (PE-paced reads, 2.9us each)
            for c in range(n_cores):
                for t in range(AT):
                    xts = xts_pool.tile([P, KT, P], F16, name="xts")
                    nc.sync.dma_start(xts[:], xg[c, :, t, :, :])
                    mm_tile(xts[:], c * SL + t * P)
            # remaining B-tiles
            for g in range(4, NBT // 2):
                for t in range(2):
                    mm_b(g, t)

    nc.compile()
    nc.finalize()
    return nc


_CACHE = {}
TRACE = False
LAST_EXEC_NS = None
LAST_RES = None


def _get_nc():
    if "nc" not in _CACHE:
        _CACHE["nc"] = build_bass(4096, 4096, 512)
    return _CACHE["nc"]


def _probe_check(out, xf, codes, absmax, bias, rng):
    """Cheap host check: one random bs row per core shard vs exact math."""
    BS, IN = xf.shape
    OSH = out.shape[1] // N_CORES
    scale = np.repeat(absmax.astype(np.float64), BLOCKSIZE, axis=1)
    for c in range(N_CORES):
        r = int(rng.integers(0, BS))
        osl = slice(c * OSH, (c + 1) * OSH)
        w = NF4[codes[osl]] * scale[osl]          # [OSH, IN] f64
        exp = w @ xf[r].astype(np.float64) + bias[osl]
        err = np.abs(out[r, osl] - exp).max()
        if not (err < 5.0):  # catches NaN too
            return False, c, err
    return True, -1, 0.0


def kernel(x, codes, absmax, bias):
    x = np.ascontiguousarray(np.asarray(x, dtype=np.float32))
    codes = np.ascontiguousarray(np.asarray(codes, dtype=np.int32))
    absmax = np.ascontiguousarray(np.asarray(absmax, dtype=np.float32))
    bias = np.ascontiguousarray(np.asarray(bias, dtype=np.float32))

    B, S, IN = x.shape
    OUT = codes.shape[0]
    BS = B * S
    OSH = OUT // N_CORES
    xf = np.ascontiguousarray(x.reshape(BS, IN))

    nc = _get_nc()
    SL = BS // N_CORES
    HA = SL // 2
    # xb: the second half of every core slice, shared by all cores
    xb = np.ascontiguousarray(
        xf.reshape(N_CORES, SL, IN)[:, HA:, :].reshape(BS // 2, IN))
    in_maps = []
    for c in range(N_CORES):
        osl = slice(c * OSH, (c + 1) * OSH)
        in_maps.append({
            "xa": np.ascontiguousarray(xf[c * SL:c * SL + HA]),
            "xb": xb,
            "codes": np.ascontiguousarray(codes[osl]),
            "absmax": np.ascontiguousarray(absmax[osl]),
            "bias": np.ascontiguousarray(bias[osl]),
        })
    global LAST_EXEC_NS, LAST_RES
    rng = np.random.default_rng(0)
    out = None
    for attempt in range(3):
        res = run_bass_kernel_spmd(nc, in_maps, core_ids=list(range(N_CORES)),
                                   trace=TRACE)
        LAST_EXEC_NS = res.exec_time_ns
        LAST_RES = res
        out = np.concatenate([res.results[c]["out"] for c in range(N_CORES)],
                             axis=1)
        ok, badcore, err = _probe_check(out, xf, codes, absmax, bias, rng)
        if ok:
            break
        print(f"kernel: probe check failed (core {badcore}, err {err:.1f}); "
              f"retrying ({attempt + 1}/3)", file=sys.stderr)
    return np.ascontiguousarray(out.reshape(B, S, OUT).astype(np.float32))
